# revision 2
# baseline (speedup 1.0000x reference)
"""MoE-over-image Trainium2 kernel (nn_MoEImage).

Data-parallel over batch: 8 cores x 4 samples. Per core, samples are
processed as 2 pairs; a pair occupies the two 64-partition halves of
SBUF/PSUM.

Pipeline per core:
  Phase A: features = gelu(fe_w @ x) per 2048-col chunk. The feature
           matmul uses double-bf16 weights (hi+lo accumulated in PSUM)
           so the pooled gating features are accurate enough for stable
           top-2 routing (score margins are ~4e-3). The gating pooled
           sum is harvested for free via the ACT accumulator output.
  Gating:  tiny MLP on [feat, sample]-oriented tiles, top-2 via
           vector.max_with_indices, softmax-of-2 via tanh, per-sample
           expert weights gathered with gpsimd.indirect_copy.
  Phase B: per 512-px chunk (sample pair): one 4-bank PSUM group G
           holds [sharedA | sharedB | routedA | routedB] (each K=64,
           M=128 matmul). ONE N=2048 gelu drains the whole group to
           SBUF bf16. A K=128 combine matmul folds
           0.5*(s0+s1) + w1*e1 + w2*e2 and writes into G's bank 3
           (already consumed), so 2 groups double-buffer in the 8 PSUM
           banks. Combine is software-pipelined one chunk behind so
           the PE never blocks the ACT stream. DVE copies PSUM->SBUF,
           DMA out.

The kernel is ACT(gelu)-throughput-bound: ~21M gelu elements/core.
"""

import os
import numpy as np

B, CIN, H, W = 32, 64, 128, 128
HID, OUT, E, S, RR = 64, 64, 8, 2, 16
TOP_K = 2
TEMP = 2.0
BN_EPS = 1e-5

NCORES = 8
BPC = B // NCORES          # samples per core = 4
NPAIR = BPC // 2           # 2
HWT = H * W                # 16384
CHUNK_A = 2048
NCH_A = HWT // CHUNK_A     # 8
CHUNK_B = 512
NCH_B = HWT // CHUNK_B     # 32

_CACHE = {}
LAST_RESULTS = None


def _build_program(loop_iters: int = 1, bz: bool = True):
    import concourse.bass as bass
    import concourse.mybir as mybir
    from concourse import bacc
    from concourse.tile import TileContext

    f32 = mybir.dt.float32
    bf16 = mybir.dt.bfloat16
    u16 = mybir.dt.uint16
    u32 = mybir.dt.uint32
    GELU = mybir.ActivationFunctionType.Gelu
    TANH = mybir.ActivationFunctionType.Tanh

    nc = bacc.Bacc(
        "TRN2",
        target_bir_lowering=False,
        debug=False,
        enable_asserts=False,
        num_devices=NCORES,
    )

    # ---- DRAM I/O ----
    x_d = nc.dram_tensor("x", [BPC, CIN, HWT], bf16, kind="ExternalInput").ap()
    y_d = nc.dram_tensor("y", [BPC, OUT, HWT], f32, kind="ExternalOutput").ap()

    def inp(name, shape, dt=None):
        return nc.dram_tensor(name, shape, dt or f32, kind="ExternalInput").ap()

    wf_d = inp("wf", [128, 128], bf16)       # [fe_w.T hi | fe_w.T lo], dup rows
    ws_d = inp("ws", [128, 128], bf16)       # [s0.T | s1.T] duplicated rows
    we_d = inp("we", [128, E * 64], bf16)    # expert e .T at cols e*64, dup rows
    feb_d = inp("feb", [128, 1])
    sbias_d = inp("sbias", [128, 1])
    ebt_d = inp("ebt", [128, E])       # e_b[e, p%64]
    cs_d = inp("cs", [128, 64], bf16)        # 0.5*[I;I]
    ist_d = inp("ist", [128, 64])      # [I;I]
    cidx_d = inp("cidx", [128, 8])     # (p%16) + 16j - 64*(j>=4)
    ssel_d = inp("ssel", [4, 4 * 128])  # per-sample broadcast selectors
    l1_d = inp("l1", [64, 128])
    b1_d = inp("b1", [128, 1])
    ca1_d = inp("ca1", [128, 8])
    bca1_d = inp("bca1", [8, 1])
    ca2_d = inp("ca2", [8, 128])
    bca2_d = inp("bca2", [128, 1])
    l2_d = inp("l2", [128, 64])
    b2_d = inp("b2", [64, 1])
    g3b_d = inp("g3b", [65, 8])

    import contextlib

    with TileContext(nc) as tc:
        with (
            tc.tile_pool(name="consts", bufs=1) as cpool,
            tc.tile_pool(name="fstore", bufs=1) as fpool,
            tc.tile_pool(name="route", bufs=1) as rpool,
            tc.tile_pool(name="work", bufs=2) as wpool,
        ):
            # ---- load constants ----
            wf = cpool.tile_from(wf_d)
            ws = cpool.tile_from(ws_d)
            we = cpool.tile_from(we_d)
            feb = cpool.tile_from(feb_d)
            sbias = cpool.tile_from(sbias_d)
            ebt = cpool.tile_from(ebt_d)
            cs = cpool.tile_from(cs_d)
            ist = cpool.tile_from(ist_d)
            cidx = cpool.tile_from(cidx_d)
            ssel = cpool.tile_from(ssel_d)
            l1 = cpool.tile_from(l1_d)
            b1 = cpool.tile_from(b1_d)
            ca1 = cpool.tile_from(ca1_d)
            bca1 = cpool.tile_from(bca1_d)
            ca2 = cpool.tile_from(ca2_d)
            bca2 = cpool.tile_from(bca2_d)
            l2 = cpool.tile_from(l2_d)
            b2 = cpool.tile_from(b2_d)
            g3b = cpool.tile_from(g3b_d)

            loop_cm = (tc.For_i(0, loop_iters, 1,
                                hint_engines=tuple(mybir.ALL_ENGINES))
                       if loop_iters > 1 else contextlib.nullcontext())
            with loop_cm:
                _kernel_body(nc, tc, mybir, cpool, fpool, rpool, wpool, x_d,
                             y_d, wf, ws, we, feb, sbias, ebt, cs, ist, cidx,
                             ssel, l1, b1, ca1, bca1, ca2, bca2, l2, b2, g3b,
                             GELU, TANH, f32, bf16, u16, u32, bz)

    nc.compile()
    return nc


def _kernel_body(nc, tc, mybir, cpool, fpool, rpool, wpool, x_d, y_d, wf, ws,
                 we, feb, sbias, ebt, cs, ist, cidx, ssel, l1, b1, ca1, bca1,
                 ca2, bca2, l2, b2, g3b, GELU, TANH, f32, bf16, u16, u32, bz):
            # features store per pair: [128, HWT] bf16 (32KB/partition each)
            fstore = []
            for p in range(NPAIR):
                ft = fpool.tile([128, HWT], bf16, tag=f"F{p}", name=f"F{p}")
                fstore.append(ft)

            gfp = []  # per-pair pooled feature sums [128,1]
            # ---------------- Phase A: features + pooled sum ----------------
            # Double-bf16 feature weights: wf cols 0:64 = hi, 64:128 = lo.
            # Both accumulate into the same PSUM so the (systematic) weight
            # rounding error drops to ~2^-17 — the pooled gating features
            # must resolve score margins of ~4e-3.
            with tc.tile_pool(name="psumA", bufs=2, space="PSUM") as pA:
                for p in range(NPAIR):
                    gfacc = rpool.tile([128, NCH_A], f32, tag=f"gfacc{p}",
                                       name=f"gfacc{p}")
                    for j in range(NCH_A):
                        ca = j * CHUNK_A
                        xt = wpool.tile([128, CHUNK_A], bf16, tag="xin",
                                        name=f"xt_{p}_{j}")
                        nc.sync.dma_start(
                            out=xt[0:64, :], in_=x_d[2 * p][:, ca:ca + CHUNK_A])
                        nc.sync.dma_start(
                            out=xt[64:128, :],
                            in_=x_d[2 * p + 1][:, ca:ca + CHUNK_A])
                        pf = pA.tile([128, CHUNK_A], f32, tag="pf",
                                     name=f"pf_{p}_{j}")
                        for h in range(0, CHUNK_A, 512):
                            nc.tensor.matmul(
                                pf[0:64, h:h + 512],
                                lhsT=wf[0:64, 0:64],
                                rhs=xt[0:64, h:h + 512],
                                start=True, stop=False)
                            nc.tensor.matmul(
                                pf[0:64, h:h + 512],
                                lhsT=wf[0:64, 64:128],
                                rhs=xt[0:64, h:h + 512],
                                start=False, stop=True)
                            nc.tensor.matmul(
                                pf[64:128, h:h + 512],
                                lhsT=wf[64:128, 0:64],
                                rhs=xt[64:128, h:h + 512],
                                start=True, stop=False)
                            nc.tensor.matmul(
                                pf[64:128, h:h + 512],
                                lhsT=wf[64:128, 64:128],
                                rhs=xt[64:128, h:h + 512],
                                start=False, stop=True)
                        nc.scalar.activation(
                            fstore[p][:, ca:ca + CHUNK_A], pf, GELU,
                            bias=feb, accum_out=gfacc[:, j:j + 1])
                    g = rpool.tile([128, 1], f32, tag=f"gfp{p}", name=f"gfp{p}")
                    nc.vector.reduce_sum(g, gfacc, axis=mybir.AxisListType.X)
                    gfp.append(g)

            # ---------------- Gating ----------------
            with tc.tile_pool(name="psumG", bufs=1, space="PSUM") as pG:
                # gf as [64(hid), 4(sample)]
                gft = rpool.tile([64, 4], f32, tag="gft")
                for p in range(NPAIR):
                    nc.vector.tensor_copy(gft[:, 2 * p:2 * p + 1],
                                          gfp[p][0:64, 0:1])
                    nc.sync.dma_start(out=gft[:, 2 * p + 1:2 * p + 2],
                                      in_=gfp[p][64:128, 0:1])

                ph1 = pG.tile([128, 4], f32, tag="ph1")
                nc.tensor.matmul(ph1, lhsT=l1, rhs=gft, start=True, stop=True)
                h1t = rpool.tile([128, 4], f32, tag="h1t")
                nc.scalar.activation(h1t, ph1, GELU, bias=b1)

                pa1 = pG.tile([8, 4], f32, tag="pa1")
                nc.tensor.matmul(pa1, lhsT=ca1, rhs=h1t, start=True, stop=True)
                a1 = rpool.tile([8, 4], f32, tag="a1")
                nc.scalar.activation(a1, pa1, GELU, bias=bca1)

                patt = pG.tile([128, 4], f32, tag="patt")
                nc.tensor.matmul(patt, lhsT=ca2, rhs=a1, start=True, stop=True)
                # sigmoid(2*att) = 0.5 + 0.5*tanh(att);  att = patt + bca2
                att_t = rpool.tile([128, 4], f32, tag="att_t")
                nc.scalar.activation(att_t, patt, TANH, bias=bca2)
                gate = rpool.tile([128, 4], f32, tag="gate")
                nc.vector.tensor_scalar(
                    gate, att_t, 0.5, scalar2=0.5,
                    op0=mybir.AluOpType.mult, op1=mybir.AluOpType.add)
                h1m = rpool.tile([128, 4], f32, tag="h1m")
                nc.vector.tensor_mul(h1m, h1t, gate)

                phh = pG.tile([64, 4], f32, tag="phh")
                nc.tensor.matmul(phh, lhsT=l2, rhs=h1m, start=True, stop=True)
                hhx = rpool.tile([65, 4], f32, tag="hhx")
                nc.vector.memset(hhx[64:65, :], 1.0)
                nc.scalar.activation(hhx[0:64, :], phh, GELU, bias=b2)

                psc = pG.tile([4, 8], f32, tag="psc")
                nc.tensor.matmul(psc, lhsT=hhx, rhs=g3b, start=True, stop=True)
                scores = rpool.tile([4, 8], f32, tag="scores")
                nc.vector.tensor_copy(scores, psc)

                vals = rpool.tile([4, 8], f32, tag="vals")
                idxs = rpool.tile([4, 8], u32, tag="idxs")
                nc.vector.max_with_indices(vals, idxs, scores)

                dv = rpool.tile([4, 1], f32, tag="dv")
                nc.vector.tensor_sub(dv, vals[:, 0:1], vals[:, 1:2])
                th = rpool.tile([4, 1], f32, tag="th")
                nc.scalar.activation(th, dv, TANH, scale=1.0 / (2.0 * TEMP))
                # u columns: [i1, i2, w1, w2]
                u = rpool.tile([4, 4], f32, tag="u")
                nc.vector.tensor_copy(u[:, 0:1], idxs[:, 0:1])
                nc.vector.tensor_copy(u[:, 1:2], idxs[:, 1:2])
                nc.vector.tensor_scalar(
                    u[:, 2:3], th, 0.5, scalar2=0.5,
                    op0=mybir.AluOpType.mult, op1=mybir.AluOpType.add)
                nc.vector.tensor_scalar(
                    u[:, 3:4], u[:, 2:3], -1.0, scalar2=1.0,
                    op0=mybir.AluOpType.mult, op1=mybir.AluOpType.add)

                # per-sample routing data
                wsel = []
                crw = []
                ebias = []
                for b in range(BPC):
                    pbc = pG.tile([128, 4], f32, tag="pbc")
                    nc.tensor.matmul(
                        pbc, lhsT=ssel[:, b * 128:(b + 1) * 128], rhs=u,
                        start=True, stop=True)
                    bc = rpool.tile([128, 4], f32, tag=f"bc{b}", name=f"bc{b}")
                    nc.vector.tensor_copy(bc, pbc)

                    # combine weights lhsT: [w1*I; w2*I]
                    wm = rpool.tile([128, 1], f32, tag=f"wm{b}", name=f"wm{b}")
                    nc.vector.tensor_copy(wm[0:64, :], bc[0:64, 2:3])
                    nc.vector.tensor_copy(wm[64:128, :], bc[64:128, 3:4])
                    cr = rpool.tile([128, 64], bf16, tag=f"cr{b}", name=f"cr{b}")
                    nc.vector.tensor_mul(cr, ist, wm.to_broadcast((128, 64)))
                    crw.append(cr)

                    # gather indices for expert weight columns
                    idxf = rpool.tile([128, 8], f32, tag="idxf")
                    s1 = rpool.tile([128, 1], f32, tag="s1c")
                    s2 = rpool.tile([128, 1], f32, tag="s2c")
                    nc.vector.tensor_scalar_mul(s1, bc[:, 0:1], 64.0)
                    nc.vector.tensor_scalar_mul(s2, bc[:, 1:2], 64.0)
                    nc.vector.tensor_add(idxf[:, 0:4], cidx[:, 0:4],
                                         s1.to_broadcast((128, 4)))
                    nc.vector.tensor_add(idxf[:, 4:8], cidx[:, 4:8],
                                         s2.to_broadcast((128, 4)))
                    idxu = rpool.tile([128, 8], u16, tag=f"idxu{b}",
                                      name=f"idxu{b}")
                    nc.vector.tensor_copy(idxu, idxf)
                    wsb = rpool.tile([128, 128], bf16, tag=f"wsel{b}",
                                     name=f"wsel{b}")
                    nc.gpsimd.indirect_copy(wsb, data=we, idxs=idxu,
                                            i_know_ap_gather_is_preferred=True)
                    wsel.append(wsb)

                    # expert bias gather: [e_b[i1]; e_b[i2]]
                    ebf = rpool.tile([128, 1], f32, tag="ebf")
                    nc.vector.tensor_copy(ebf[0:64, :], bc[0:64, 0:1])
                    nc.vector.tensor_copy(ebf[64:128, :], bc[64:128, 1:2])
                    ebu = rpool.tile([128, 1], u16, tag=f"ebu{b}",
                                     name=f"ebu{b}")
                    nc.vector.tensor_copy(ebu, ebf)
                    ebb = rpool.tile([128, 1], f32, tag=f"ebias{b}",
                                     name=f"ebias{b}")
                    nc.gpsimd.indirect_copy(ebb, data=ebt, idxs=ebu,
                                            i_know_ap_gather_is_preferred=True)
                    ebias.append(ebb)

            # ---------------- Phase B ----------------
            # Per 512-px chunk: one 4-bank PSUM group G =
            # [sharedA | sharedB | routedA | routedB], one N=2048 gelu,
            # K=128 combine written into G's bank 3, DVE copy, DMA out.
            # Combine is software-pipelined one chunk behind.
            with tc.tile_pool(name="psumB", bufs=2, space="PSUM") as pB:
                for p in range(NPAIR):
                    F = fstore[p]
                    crA, crB = crw[2 * p], crw[2 * p + 1]
                    wsA, wsB = wsel[2 * p], wsel[2 * p + 1]
                    eb0, eb1 = ebias[2 * p], ebias[2 * p + 1]
                    pend = None  # (G, g, i) awaiting combine
                    for i in range(NCH_B):
                        cc = i * CHUNK_B
                        rt = F[0:64, cc:cc + CHUNK_B]
                        rb = F[64:128, cc:cc + CHUNK_B]
                        G = pB.tile([128, 4 * CHUNK_B], f32, tag="G",
                                    name=f"G_{p}_{i}")
                        nc.tensor.matmul(G[:, 0:512], lhsT=ws[0:64, :],
                                         rhs=rt, start=True, stop=True)
                        nc.tensor.matmul(G[:, 512:1024], lhsT=ws[64:128, :],
                                         rhs=rb, start=True, stop=True)
                        nc.tensor.matmul(G[:, 1024:1536], lhsT=wsA[0:64, :],
                                         rhs=rt, start=True, stop=True)
                        nc.tensor.matmul(G[:, 1536:2048], lhsT=wsB[64:128, :],
                                         rhs=rb, start=True, stop=True)
                        g = wpool.tile([128, 4 * CHUNK_B], bf16, tag="g",
                                       name=f"g_{p}_{i}")
                        if bz:
                            nc.scalar.activation(g, G, GELU)
                        else:
                            nc.scalar.activation(g[:, 0:1024], G[:, 0:1024],
                                                 GELU, bias=sbias)
                            nc.scalar.activation(g[:, 1024:1536],
                                                 G[:, 1024:1536], GELU,
                                                 bias=eb0)
                            nc.scalar.activation(g[:, 1536:2048],
                                                 G[:, 1536:2048], GELU,
                                                 bias=eb1)

                        if pend is not None:
                            _emit_combine(nc, wpool, y_d, cs, crA, crB,
                                          p, *pend, f32)
                        pend = (G, g, i)
                    _emit_combine(nc, wpool, y_d, cs, crA, crB, p, *pend, f32)


def _emit_combine(nc, wpool, y_d, cs, crA, crB, p, G, g, i, f32):
    cc = i * CHUNK_B
    po = G[:, 3 * CHUNK_B:4 * CHUNK_B]  # bank 3, already drained by ACT
    nc.tensor.matmul(po[0:64, :], lhsT=cs, rhs=g[:, 0:512],
                     start=True, stop=False)
    nc.tensor.matmul(po[0:64, :], lhsT=crA, rhs=g[:, 1024:1536],
                     start=False, stop=True)
    nc.tensor.matmul(po[64:128, :], lhsT=cs, rhs=g[:, 512:1024],
                     start=True, stop=False)
    nc.tensor.matmul(po[64:128, :], lhsT=crB, rhs=g[:, 1536:2048],
                     start=False, stop=True)
    ost = wpool.tile([128, CHUNK_B], f32, tag="ost", name=f"ost_{p}_{i}")
    nc.vector.tensor_copy(ost, po)
    nc.sync.dma_start(out=y_d[2 * p][:, cc:cc + CHUNK_B], in_=ost[0:64, :])
    nc.sync.dma_start(out=y_d[2 * p + 1][:, cc:cc + CHUNK_B],
                      in_=ost[64:128, :])


def _host_consts(fe_w, fe_b, s_w, s_b, e_w, e_b, g1_w, g1_b, bn1_g, bn1_b,
                 ca1_w, ca1_b, ca2_w, ca2_b, g2_w, g2_b, bn2_g, bn2_b,
                 g3_w, g3_b):
    import ml_dtypes
    bf = ml_dtypes.bfloat16
    f = np.float32
    I64 = np.eye(64, dtype=f)

    def dup(a):  # duplicate along partition dim
        return np.concatenate([a, a], axis=0).astype(f)

    # double-bf16 split of the feature weights (routing-critical)
    wft = fe_w.T.astype(f)
    wf_hi = wft.astype(bf)
    wf_lo = (wft - wf_hi.astype(f)).astype(bf)
    wf = dup(np.concatenate([wf_hi.astype(f), wf_lo.astype(f)], axis=1))

    ws = dup(np.concatenate([s_w[0].T, s_w[1].T], axis=1))   # [128,128]
    we = dup(np.concatenate([e_w[e].T for e in range(E)], axis=1))  # [128,512]
    feb = np.concatenate([fe_b, fe_b]).reshape(128, 1).astype(f)
    sbias = np.concatenate([s_b[0], s_b[1]]).reshape(128, 1).astype(f)
    ebt = np.concatenate([e_b.T, e_b.T], axis=0).astype(f)   # [128, E]
    cs = 0.5 * np.concatenate([I64, I64], axis=0)
    ist = np.concatenate([I64, I64], axis=0)

    pm = np.arange(128) % 16
    jj = np.arange(8)
    cidx = (pm[:, None] + 16 * jj[None, :] - 64 * (jj[None, :] >= 4)).astype(f)

    ssel = np.zeros((4, 4 * 128), dtype=f)
    for b in range(4):
        ssel[b, b * 128:(b + 1) * 128] = 1.0

    s1 = (bn1_g / np.sqrt(1.0 + BN_EPS)).astype(f)
    l1 = ((g1_w * s1[:, None]) / float(HWT)).T.astype(f)     # [64, 128]
    b1 = (g1_b * s1 + bn1_b).reshape(128, 1).astype(f)
    ca1 = ca1_w.T.astype(f)                                  # [128, 8]
    bca1 = ca1_b.reshape(8, 1).astype(f)
    ca2 = ca2_w.T.astype(f)                                  # [8, 128]
    bca2 = ca2_b.reshape(128, 1).astype(f)
    s2 = (bn2_g / np.sqrt(1.0 + BN_EPS)).astype(f)
    l2 = (g2_w * s2[:, None]).T.astype(f)                    # [128, 64]
    b2 = (g2_b * s2 + bn2_b).reshape(64, 1).astype(f)
    g3b = np.concatenate([g3_w.T, g3_b.reshape(1, 8)], axis=0).astype(f)

    return dict(wf=wf, ws=ws, we=we, feb=feb, sbias=sbias, ebt=ebt, cs=cs,
                ist=ist, cidx=cidx, ssel=ssel, l1=l1, b1=b1, ca1=ca1,
                bca1=bca1, ca2=ca2, bca2=bca2, l2=l2, b2=b2, g3b=g3b)


def kernel(**inputs):
    global LAST_RESULTS
    import sys
    if "/opt/trn_rl_repo" not in sys.path:
        sys.path.insert(0, "/opt/trn_rl_repo")
    from concourse import bass_utils

    import ml_dtypes
    bf = ml_dtypes.bfloat16
    x = np.ascontiguousarray(np.asarray(inputs["x"], dtype=np.float32).astype(bf))
    consts = _host_consts(**{k: np.asarray(v, np.float32)
                             for k, v in inputs.items() if k != "x"})
    for k in ("wf", "ws", "we", "cs"):
        consts[k] = consts[k].astype(bf)

    bz = bool(np.all(np.asarray(inputs["e_b"]) == 0.0)
              and np.all(np.asarray(inputs["s_b"]) == 0.0))
    key = ("nc", bz)
    if key not in _CACHE:
        _CACHE[key] = _build_program(1, bz)
    nc = _CACHE[key]

    xr = x.reshape(B, CIN, HWT)
    in_maps = []
    for c in range(NCORES):
        m = {"x": np.ascontiguousarray(xr[c * BPC:(c + 1) * BPC])}
        m.update(consts)
        in_maps.append(m)

    trace = bool(int(os.environ.get("MOE_KERNEL_TRACE", "0")))
    res = bass_utils.run_bass_kernel_spmd(
        nc, in_maps, core_ids=list(range(NCORES)), trace=trace)
    LAST_RESULTS = res
    out = np.concatenate([r["y"] for r in res.results], axis=0)
    return out.reshape(B, OUT, H, W)
